# revision 26
# baseline (speedup 1.0000x reference)
"""Trainium2 Bass kernel for the Mamba encoder block.

Sharding: 8 cores = (batch b in 0..3) x (SSM-state-half in 0..1).
Each pair of cores duplicates the token-wise "pre" stages (LN folded into the
in-projection matmul, causal depthwise conv as PE diag-matmuls, delta/softplus),
splits the 16 SSM states 8/8 (scan via the native DVE tensor_tensor_scan,
vectorized over d=128 on partitions, chained over t-chunks), merges the
out-projection partials with one pair AllReduce, then both cores run the
duplicated post stage (axial depthwise convs as DVE/Pool shifted-madd taps,
pointwise conv on PE, BatchNorm with a tiny 8-core stats AllReduce, ReLU,
2x2 maxpool).

The Mamba pipeline runs in bf16 on PE/DVE (fp32 accumulation everywhere);
the axial/pointwise tail stays fp32. Bs/Cs row broadcasts are done by DMA
with a 0-step free axis. All per-core asymmetry is carried by input data so
the SPMD program is identical on every core.
"""
import sys
import numpy as np

sys.path.insert(0, '/opt/trn_rl_repo')

import concourse.bass as bass          # noqa: E402
import concourse.bacc as bacc          # noqa: E402
import concourse.tile as tile          # noqa: E402
from concourse import mybir            # noqa: E402
from concourse import bass_utils       # noqa: E402
import ml_dtypes                       # noqa: E402

F32 = mybir.dt.float32
BF16 = mybir.dt.bfloat16
F32R = mybir.dt.float32r
AF = mybir.ActivationFunctionType
OP = mybir.AluOpType

B, C, H, W = 4, 64, 128, 128
L = H * W                  # 16384
DI, N, DCONV, DTR = 128, 16, 4, 4
EPS = 1e-5
TC = 2048                  # tokens per chunk
NCH = L // TC              # 8 chunks
SUB = 512                  # matmul / PSUM granularity
NSUB = TC // SUB
SC = 1024                  # scan sub-chain length
NLOC = N // 2              # states per core
CNT = 2.0 * B * H * W      # BN count incl. pair duplication
RPC = 16                   # H rows per post-chunk
NPC = H // RPC

_prog_cache = {}

# Force a single ACT table set (contains Exp/Ln/Square/Identity/Copy/Relu) so
# Bacc never inserts mid-kernel table swaps (~1.3us each).
import concourse.hw_specs as _hw
_orig_gat = _hw.get_activation_tables
def _one_set_tables(arch):
    t = _orig_gat(arch)
    keep = "natural_log_exp_and_others"
    return {k: (v if k == keep else set()) for k, v in t.items()}



def _bcast_dma(nc, dst, src_row):
    """DMA-broadcast a [1, f] SBUF row to a [p, f] SBUF tile via 0-step free axis."""
    src = bass.AP(tensor=src_row.tensor, offset=src_row.offset,
                  ap=[src_row.ap[0], [0, dst.shape[0]], src_row.ap[1]])
    d = dst[:]
    dst2 = bass.AP(tensor=d.tensor, offset=d.offset,
                   ap=[d.ap[0], [0, 1], d.ap[1]])
    nc.sync.dma_start(dst2, src)


def _build_program():
    _hw.get_activation_tables.__wrapped__  # ensure cache attr exists
    _hw.get_activation_tables = _one_set_tables
    import concourse.bacc as _bacc_mod
    _bacc_mod.get_activation_tables = _one_set_tables
    nc = bacc.Bacc()

    t = {}
    def di(name, shape, dt=F32):
        t[name] = nc.dram_tensor(name, shape, dt, kind="ExternalInput")

    di("xb", [C, L])
    di("lhsT_xz", [C + 1, 2 * DI], BF16)
    di("ones64", [C, C], BF16)
    di("conv_diag", [DI, DCONV * DI], BF16)
    di("conv_be", [DI, 1])
    di("lhsT_xp", [DI, 72], BF16)
    di("lhsT_dt", [DTR, DI], BF16)
    di("b_dt", [DI, 1])
    di("A_cols", [DI, NLOC])
    di("diag_D", [DI, DI], BF16)
    di("lhsT_out", [DI, C], BF16)
    di("I128", [DI, DI], BF16)
    di("sel8", [72, NLOC * DI], BF16)
    di("ax5", [C, 5 * C], BF16)
    di("bias_ax", [C, 1])
    di("lhsT_pw", [C, 128])
    di("pw_b", [128, 1])
    di("bn_g", [128, 1])
    di("bn_b", [128, 1])

    t["pooled_o"] = nc.dram_tensor("pooled_o", [128, H // 2, W // 2], F32,
                                   kind="ExternalOutput")
    t["skip_o"] = nc.dram_tensor("skip_o", [C, L], F32, kind="ExternalOutput")

    with nc.allow_low_precision("f32r tail is ~1e-4 accurate, verified"), \
         tile.TileContext(nc, num_cores=8) as tc:
        _body(nc, tc, t)
    nc.finalize()
    return nc


def _body(nc, tc, t):
    xb, pooled_o, skip_o = t['xb'], t['pooled_o'], t['skip_o']

    import contextlib
    ctx = contextlib.ExitStack()
    with ctx:
        singles = ctx.enter_context(tc.tile_pool(name="singles", bufs=1))
        dram = ctx.enter_context(tc.tile_pool(name="dram", bufs=1, space="DRAM"))

        def const(name, shape, dt=F32):
            tl = singles.tile(shape, dt, tag=name, name=name + "_c")
            nc.sync.dma_start(tl[:], t[name][:])
            return tl

        c_xz = const("lhsT_xz", [C + 1, 2 * DI], BF16)
        c_o64 = const("ones64", [C, C], BF16)
        c_cd = const("conv_diag", [DI, DCONV * DI], BF16)
        c_cbe = const("conv_be", [DI, 1])
        c_xp = const("lhsT_xp", [DI, 72], BF16)
        c_dt = const("lhsT_dt", [DTR, DI], BF16)
        c_bdt = const("b_dt", [DI, 1])
        c_A = const("A_cols", [DI, NLOC])
        c_dD = const("diag_D", [DI, DI], BF16)
        c_out = const("lhsT_out", [DI, C], BF16)
        c_I = const("I128", [DI, DI], BF16)
        c_sel = const("sel8", [72, NLOC * DI], BF16)
        c_ax5 = const("ax5", [C, 5 * C], BF16)
        c_bax = const("bias_ax", [C, 1])
        c_pw = const("lhsT_pw", [C, 128])
        c_pwb = const("pw_b", [128, 1])
        c_bng = const("bn_g", [128, 1])
        c_bnb = const("bn_b", [128, 1])

        carries = []
        for n in range(NLOC):
            cn = singles.tile([DI, 1], F32, tag=f"carry{n}", name=f"carry{n}")
            nc.vector.memset(cn[:], 0.0)
            carries.append(cn)
        sig1 = singles.tile([C, NPC], F32, tag="sig1")
        s2ac = singles.tile([128, NPC * NSUB], F32, tag="s2ac")
        c_eps = singles.tile([DI, 1], F32, tag="c_eps")
        nc.vector.memset(c_eps[:], EPS)
        c_one = singles.tile([DI, 1], F32, tag="c_one")
        nc.vector.memset(c_one[:], 1.0)

        cc_in_a = dram.tile([C, L // 2], F32, tag="cc_in_a")
        cc_in_b = dram.tile([C, L // 2], F32, tag="cc_in_b")
        cc_out_a = dram.tile([C, L // 2], F32, tag="cc_out_a")
        cc_out_b = dram.tile([C, L // 2], F32, tag="cc_out_b")

        def silu_chain(src_ap, bias_ap, out_ap, wpool, p, f):
            """out = silu(src + bias), reciprocal-free: sigmoid(v)=exp(-softplus(-v))."""
            v = wpool.tile([p, f], BF16, tag="scrb", name="sv")
            if bias_ap is None:
                nc.scalar.activation(out=v[:], in_=src_ap, func=AF.Copy)
            else:
                nc.scalar.activation(out=v[:], in_=src_ap, func=AF.Identity, bias=bias_ap)
            e = wpool.tile([p, f], BF16, tag="scrb", name="se")
            nc.scalar.activation(out=e[:], in_=v[:], func=AF.Exp, scale=-1.0)
            sg = wpool.tile([p, f], BF16, tag="scrb", name="ssg")
            nc.scalar.activation(out=sg[:], in_=e[:], func=AF.Ln, bias=c_one[0:p, :])
            nc.scalar.activation(out=sg[:], in_=sg[:], func=AF.Exp, scale=-1.0)
            nc.vector.tensor_mul(out=out_ap, in0=v[:], in1=sg[:])

        # ================= PRE + SCAN (per t-chunk) =================
        with tc.tile_pool(name="bigp", bufs=2) as bp, \
             tc.tile_pool(name="dblp", bufs=2) as dp, \
             tc.tile_pool(name="scanp", bufs=4) as sp, \
             tc.tile_pool(name="workp", bufs=3) as wp, \
             tc.tile_pool(name="ps", bufs=4, space="PSUM") as pp, \
             tc.tile_pool(name="psy", bufs=4, space="PSUM") as ppy:

            prev_xmpre = None
            for ci in range(NCH):
                t0 = ci * TC
                x_bf = bp.tile([C, TC], BF16, tag="x_bf")
                nc.gpsimd.dma_start(x_bf[:], xb[:, t0:t0 + TC])   # f32 -> bf16 cast

                xm_pre = bp.tile([DI, TC + 3], BF16, tag="xm_pre")
                if ci == 0:
                    nc.vector.memset(xm_pre[:, 0:3], 0.0)
                else:
                    nc.vector.tensor_copy(out=xm_pre[:, 0:3],
                                          in_=prev_xmpre[:, TC:TC + 3])
                z_t = bp.tile([DI, TC], BF16, tag="z_t")

                xq = bp.tile([C + 1, TC], BF16, tag="xq")   # normalized x + ones row
                nc.gpsimd.memset(xq[C:C + 1, :], 1.0)
                for s in range(NSUB):
                    sl = slice(s * SUB, (s + 1) * SUB)
                    sq = wp.tile([C, SUB], BF16, tag="sqb")
                    nc.scalar.activation(out=sq[:], in_=x_bf[:, sl], func=AF.Square)
                    mu = pp.tile([C, SUB], F32, tag="w")
                    nc.tensor.matmul(mu[:], c_o64[:], x_bf[:, sl], start=True, stop=True)
                    msq = pp.tile([C, SUB], F32, tag="w")
                    nc.tensor.matmul(msq[:], c_o64[:], sq[:], start=True, stop=True)
                    musq = wp.tile([C, SUB], F32, tag="scr")
                    nc.scalar.activation(out=musq[:], in_=mu[:], func=AF.Square)
                    var = wp.tile([C, SUB], F32, tag="scr")
                    nc.vector.tensor_sub(out=var[:], in0=msq[:], in1=musq[:])
                    lnv = wp.tile([C, SUB], F32, tag="scr")
                    nc.scalar.activation(out=lnv[:], in_=var[:], func=AF.Ln, bias=c_eps[0:C, :])
                    rstd = wp.tile([C, SUB], F32, tag="rstd")
                    nc.scalar.activation(out=rstd[:], in_=lnv[:], func=AF.Exp, scale=-0.5)
                    xc = wp.tile([C, SUB], F32, tag="scr")
                    nc.vector.tensor_sub(out=xc[:], in0=x_bf[:, sl], in1=mu[:])
                    nc.vector.tensor_mul(out=xq[0:C, sl], in0=xc[:], in1=rstd[:])

                    mmx = pp.tile([DI, SUB], F32, tag="w")
                    nc.tensor.matmul(mmx[:], c_xz[:, 0:DI], xq[:, sl], start=True, stop=True)
                    mmz = pp.tile([DI, SUB], F32, tag="w")
                    nc.tensor.matmul(mmz[:], c_xz[:, DI:2 * DI], xq[:, sl], start=True, stop=True)
                    xsl = slice(3 + s * SUB, 3 + (s + 1) * SUB)
                    nc.scalar.activation(out=xm_pre[:, xsl], in_=mmx[:], func=AF.Copy)
                    nc.scalar.activation(out=z_t[:, sl], in_=mmz[:], func=AF.Copy)

                # causal depthwise conv (4 taps as bf16 diag matmuls) + silu
                xm = bp.tile([DI, TC], BF16, tag="xm")
                for s in range(NSUB):
                    cps = pp.tile([DI, SUB], F32, tag="w")
                    for k in range(DCONV):
                        nc.tensor.matmul(
                            cps[:], c_cd[:, k * DI:(k + 1) * DI],
                            xm_pre[:, s * SUB + k: s * SUB + k + SUB],
                            start=(k == 0), stop=(k == DCONV - 1))
                    silu_chain(cps[:], c_cbe[:], xm[:, s * SUB:(s + 1) * SUB], wp, DI, SUB)

                # z silu -> sz (in-place into z_t)
                for s in range(NSUB):
                    sl = slice(s * SUB, (s + 1) * SUB)
                    silu_chain(z_t[:, sl], None, z_t[:, sl], wp, DI, SUB)
                sz = z_t

                # xproj -> dbl rows: dt 0:4, Bs 32:40, Cs 64:72 (bf16)
                dbl = dp.tile([72, TC], BF16, tag="dbl")
                for s in range(NSUB):
                    sl = slice(s * SUB, (s + 1) * SUB)
                    dps = pp.tile([72, SUB], F32, tag="w")
                    nc.tensor.matmul(dps[:], c_xp[:], xm[:, sl], start=True, stop=True)
                    nc.scalar.activation(out=dbl[:, sl], in_=dps[:], func=AF.Copy)

                # delta = softplus(W_dt @ dt + b_dt) = ln(1 + exp(v))   (f32)
                delta = bp.tile([DI, TC], F32, tag="delta")
                u_t = bp.tile([DI, TC], BF16, tag="u_t")
                for s in range(NSUB):
                    sl = slice(s * SUB, (s + 1) * SUB)
                    dmp = pp.tile([DI, SUB], F32, tag="w")
                    nc.tensor.matmul(dmp[:], c_dt[:], dbl[0:4, sl], start=True, stop=True)
                    ev = wp.tile([DI, SUB], F32, tag="scr")
                    nc.scalar.activation(out=ev[:], in_=dmp[:], func=AF.Exp, bias=c_bdt[:])
                    nc.scalar.activation(out=delta[:, sl], in_=ev[:], func=AF.Ln, bias=c_one[:])
                    nc.vector.tensor_mul(out=u_t[:, sl], in0=delta[:, sl], in1=xm[:, sl])

                # ------- scan over the 8 local states -------
                yps = [ppy.tile([DI, SUB], F32, tag="y", name=f"yp{ci}_{si}")
                       for si in range(NSUB)]
                for n in range(NLOC):
                    a_t = sp.tile([DI, TC], BF16, tag="a_t")
                    nc.scalar.activation(out=a_t[:], in_=delta[:], func=AF.Exp,
                                         scale=c_A[:, n:n + 1])
                    for g in range(2):
                        gl = slice(g * SC, (g + 1) * SC)
                        b_t = sp.tile([DI, SC], BF16, tag="b_t")
                        for si in range(SC // SUB):
                            fs = slice(g * SC + si * SUB, g * SC + (si + 1) * SUB)
                            bbc = pp.tile([DI, SUB], F32, tag="w")
                            nc.tensor.matmul(bbc[:], c_sel[32:40, n * DI:(n + 1) * DI],
                                             dbl[32:40, fs], start=True, stop=True)
                            nc.vector.tensor_mul(out=b_t[:, si * SUB:(si + 1) * SUB],
                                                 in0=u_t[:, fs], in1=bbc[:])
                        h_t = sp.tile([DI, SC], BF16, tag="h_t")
                        nc.vector.tensor_tensor_scan(
                            out=h_t[:], data0=a_t[:, gl], data1=b_t[:],
                            initial=carries[n][:], op0=OP.mult, op1=OP.add)
                        nc.vector.tensor_copy(out=carries[n][:],
                                              in_=h_t[:, SC - 1:SC])
                        rc = sp.tile([1, SC], BF16, tag="rowt")
                        nc.sync.dma_start(rc[:], dbl[64 + n:65 + n, gl])
                        cbc = sp.tile([DI, SC], BF16, tag="cbc")
                        nc.gpsimd.partition_broadcast(cbc[:], rc[:])
                        hc = wp.tile([DI, SC], BF16, tag="hc")
                        nc.vector.tensor_mul(out=hc[:], in0=h_t[:], in1=cbc[:])
                        for si in range(SC // SUB):
                            s = g * (SC // SUB) + si
                            nc.tensor.matmul(yps[s][:], c_I[:],
                                             hc[:, si * SUB:(si + 1) * SUB],
                                             start=(n == 0), stop=False)

                # finalize y, gate, project
                for s in range(NSUB):
                    sl = slice(s * SUB, (s + 1) * SUB)
                    nc.tensor.matmul(yps[s][:], c_dD[:], xm[:, sl], start=False, stop=True)
                    yg = wp.tile([DI, SUB], BF16, tag="yg")
                    nc.vector.tensor_mul(out=yg[:], in0=yps[s][:], in1=sz[:, sl])
                    ops = pp.tile([C, SUB], F32, tag="w")
                    nc.tensor.matmul(ops[:], c_out[:], yg[:], start=True, stop=True)
                    obuf = wp.tile([C, SUB], F32, tag="obuf")
                    nc.scalar.activation(out=obuf[:], in_=ops[:], func=AF.Copy)
                    cct = cc_in_a if ci < NCH // 2 else cc_in_b
                    toff = t0 - (0 if ci < NCH // 2 else L // 2)
                    nc.sync.dma_start(cct[:, toff + s * SUB:toff + (s + 1) * SUB], obuf[:])
                if ci == NCH // 2 - 1:
                    nc.gpsimd.collective_compute(
                        "AllReduce", OP.add,
                        replica_groups=[[0, 1], [2, 3], [4, 5], [6, 7]],
                        ins=[cc_in_a.opt()], outs=[cc_out_a.opt()])
                prev_xmpre = xm_pre

        # ================= pair AllReduce of out (2nd half) =================
        nc.gpsimd.collective_compute(
            "AllReduce", OP.add,
            replica_groups=[[0, 1], [2, 3], [4, 5], [6, 7]],
            ins=[cc_in_b.opt()], outs=[cc_out_b.opt()])

        # ================= POST pass A: axial + BN stats =================
        with tc.tile_pool(name="postw", bufs=2) as pq, \
             tc.tile_pool(name="posts", bufs=3) as ps3, \
             tc.tile_pool(name="psax", bufs=1, space="PSUM") as ppax, \
             tc.tile_pool(name="psw", bufs=4, space="PSUM") as ppw:
            for pc in range(NPC):
                r0 = pc * RPC
                lo = max(r0 - 1, 0)
                hi = min(r0 + RPC + 1, H)
                xr = pq.tile([C, RPC + 2, W + 2], BF16, tag="xr")
                nc.gpsimd.memset(xr[:, :, 0:1], 0.0)
                nc.gpsimd.memset(xr[:, :, W + 1:W + 2], 0.0)
                if pc == 0:
                    nc.gpsimd.memset(xr[:, 0:1, :], 0.0)
                if pc == NPC - 1:
                    nc.gpsimd.memset(xr[:, RPC + 1:RPC + 2, :], 0.0)
                xrows = pq.tile([C, RPC + 2, W], F32, tag="xrows")
                nc.sync.dma_start(
                    xrows[:, 0:hi - lo, :],
                    xb[:, lo * W:hi * W].rearrange("p (a b) -> p a b", b=W))
                ro = lo - (r0 - 1)   # 1 for pc=0 else 0
                orows = pq.tile([C, RPC + 2, W], F32, tag="orows")
                HB = (H // 2) * W
                if hi * W <= HB:
                    nc.sync.dma_start(
                        orows[:, 0:hi - lo, :],
                        cc_out_a[:, lo * W:hi * W].rearrange("p (a b) -> p a b", b=W))
                elif lo * W >= HB:
                    nc.sync.dma_start(
                        orows[:, 0:hi - lo, :],
                        cc_out_b[:, lo * W - HB:hi * W - HB].rearrange("p (a b) -> p a b", b=W))
                else:
                    mid = H // 2
                    nc.sync.dma_start(
                        orows[:, 0:mid - lo, :],
                        cc_out_a[:, lo * W:HB].rearrange("p (a b) -> p a b", b=W))
                    nc.sync.dma_start(
                        orows[:, mid - lo:hi - lo, :],
                        cc_out_b[:, 0:hi * W - HB].rearrange("p (a b) -> p a b", b=W))
                nc.vector.tensor_add(
                    out=xr[:, ro:ro + (hi - lo), 1:W + 1],
                    in0=orows[:, 0:hi - lo, :],
                    in1=xrows[:, 0:hi - lo, :])
                oi = r0 - lo
                xid = pq.tile([C, RPC * W], F32, tag="xid")
                xid3 = xid[:].rearrange("p (a b) -> p a b", b=W)
                nc.vector.tensor_add(
                    out=xid3,
                    in0=orows[:, oi:oi + RPC, :],
                    in1=xrows[:, oi:oi + RPC, :])
                axp = ppax.tile([C, RPC * W], F32, tag="ax")
                for s in range(NSUB):
                    rlo = 1 + s * 4
                    osl = slice(s * SUB, (s + 1) * SUB)
                    taps = [(0, rlo, 1), (1, rlo - 1, 1), (2, rlo + 1, 1),
                            (3, rlo, 0), (4, rlo, 2)]
                    for i, (kb, rr, cc0) in enumerate(taps):
                        nc.tensor.matmul(
                            axp[:, osl], c_ax5[:, kb * C:(kb + 1) * C],
                            xr[:, rr:rr + 4, cc0:cc0 + W],
                            start=(i == 0), stop=(i == len(taps) - 1))
                skp = pq.tile([C, RPC * W], F32, tag="skp")
                nc.vector.scalar_tensor_tensor(
                    out=skp[:], in0=xid[:], scalar=c_bax[0:C, :],
                    in1=axp[:], op0=OP.add, op1=OP.add,
                    accum_out=sig1[:, pc:pc + 1])
                nc.sync.dma_start(skip_o[:, r0 * W:(r0 + RPC) * W], skp[:])
                for s in range(NSUB):
                    sl = slice(s * SUB, (s + 1) * SUB)
                    pwp = ppw.tile([128, SUB], F32, tag="pw")
                    nc.tensor.matmul(pwp[:], c_pw[:], skp[:, sl], start=True, stop=True)
                    sqs = ps3.tile([128, SUB], F32, tag="sqs")
                    nc.scalar.activation(out=sqs[:], in_=pwp[:], func=AF.Square,
                                         bias=c_pwb[:],
                                         accum_out=s2ac[:, pc * NSUB + s:pc * NSUB + s + 1])

            # ---------- BN stats allreduce ----------
            st = ps3.tile([128, 2], F32, tag="st")
            s1c = ps3.tile([C, 1], F32, tag="s1c")
            nc.vector.tensor_reduce(out=s1c[:], in_=sig1[:], axis=mybir.AxisListType.X,
                                    op=OP.add)
            s1p = ppw.tile([128, 2], F32, tag="pw")
            nc.tensor.matmul(s1p[:, 0:1], c_pw[:], s1c[:], start=True, stop=True)
            nc.scalar.activation(out=st[:, 0:1], in_=s1p[:, 0:1], func=AF.Copy)
            nc.vector.tensor_reduce(out=st[:, 1:2], in_=s2ac[:], axis=mybir.AxisListType.X,
                                    op=OP.add)
            bn_in = dram.tile([128, 2], F32, tag="bn_in")
            bn_out = dram.tile([128, 2], F32, tag="bn_out")
            nc.sync.dma_start(bn_in[:], st[:])
            nc.gpsimd.collective_compute(
                "AllReduce", OP.add,
                replica_groups=[[0, 1, 2, 3, 4, 5, 6, 7]],
                ins=[bn_in.opt()], outs=[bn_out.opt()])
            stg = ps3.tile([128, 2], F32, tag="stg")
            nc.sync.dma_start(stg[:], bn_out[:])
            mean = ps3.tile([128, 1], F32, tag="mean")
            nc.scalar.activation(out=mean[:], in_=stg[:, 0:1], func=AF.Identity,
                                 scale=1.0 / CNT, bias=c_pwb[:])
            msq2 = ps3.tile([128, 1], F32, tag="msq2")
            nc.scalar.activation(out=msq2[:], in_=mean[:], func=AF.Square)
            var = ps3.tile([128, 1], F32, tag="var")
            nc.vector.scalar_tensor_tensor(out=var[:], in0=stg[:, 1:2], scalar=1.0 / CNT,
                                           in1=msq2[:], op0=OP.mult, op1=OP.subtract)
            lnv = ps3.tile([128, 1], F32, tag="lnv")
            nc.scalar.activation(out=lnv[:], in_=var[:], func=AF.Ln, bias=c_eps[:])
            rsd = ps3.tile([128, 1], F32, tag="rsd")
            nc.scalar.activation(out=rsd[:], in_=lnv[:], func=AF.Exp, scale=-0.5)
            gsc = ps3.tile([128, 1], F32, tag="gsc")
            nc.vector.tensor_mul(out=gsc[:], in0=rsd[:], in1=c_bng[:])
            pmm = ps3.tile([128, 1], F32, tag="pmm")
            nc.vector.tensor_sub(out=pmm[:], in0=c_pwb[:], in1=mean[:])
            nc.vector.tensor_mul(out=pmm[:], in0=pmm[:], in1=gsc[:])
            gbias = ps3.tile([128, 1], F32, tag="gbias")
            nc.vector.scalar_tensor_tensor(out=gbias[:], in0=pmm[:], scalar=1.0,
                                           in1=c_bnb[:], op0=OP.mult, op1=OP.add)

            # ---------- pass B: pw -> BN -> ReLU -> maxpool ----------
            for pc in range(NPC):
                r0 = pc * RPC
                skq = pq.tile([C, RPC * W], F32, tag="skq")
                nc.sync.dma_start(skq[:], skip_o[:, r0 * W:(r0 + RPC) * W])
                for s in range(NSUB):
                    sl = slice(s * SUB, (s + 1) * SUB)
                    pwp = ppw.tile([128, SUB], F32, tag="pw")
                    nc.tensor.matmul(pwp[:], c_pw[:], skq[:, sl], start=True, stop=True)
                    rl = ps3.tile([128, SUB], F32, tag="rl")
                    nc.scalar.activation(out=rl[:], in_=pwp[:], func=AF.Relu,
                                         scale=gsc[:], bias=gbias[:])
                    rl3 = rl[:].rearrange("p (a b) -> p a b", b=W)
                    wt = ps3.tile([128, 4, W // 2], F32, tag="wt")
                    nc.vector.tensor_max(out=wt[:], in0=rl3[:, :, 0:W:2], in1=rl3[:, :, 1:W:2])
                    pt = ps3.tile([128, 2, W // 2], F32, tag="pt")
                    nc.vector.tensor_max(out=pt[:], in0=wt[:, 0:4:2, :], in1=wt[:, 1:4:2, :])
                    pr0 = (r0 + s * 4) // 2
                    nc.sync.dma_start(pooled_o[:, pr0:pr0 + 2, :], pt[:])


def _sel72():
    s = np.zeros((72, NLOC * DI), np.float32)
    for n in range(NLOC):
        s[32 + n, n * DI:(n + 1) * DI] = 1.0
        s[64 + n, n * DI:(n + 1) * DI] = 1.0
    return s


def _host_prep(inputs):
    bf = ml_dtypes.bfloat16
    x = np.ascontiguousarray(np.asarray(inputs['x'], dtype=np.float32))
    ln_w = np.asarray(inputs['ln_w'], np.float32)
    ln_b = np.asarray(inputs['ln_b'], np.float32)
    W_in = np.asarray(inputs['W_in'], np.float32)
    conv_w = np.asarray(inputs['conv_w'], np.float32)[:, 0, :]
    conv_b = np.asarray(inputs['conv_b'], np.float32)
    W_xp = np.asarray(inputs['W_xproj'], np.float32)
    W_dt = np.asarray(inputs['W_dt'], np.float32)
    b_dt = np.asarray(inputs['b_dt'], np.float32)
    A = -np.exp(np.asarray(inputs['A_log'], np.float32))
    D_param = np.asarray(inputs['D_param'], np.float32)
    W_out = np.asarray(inputs['W_out'], np.float32)
    wh = np.asarray(inputs['dwh_w'], np.float32)[:, 0, :, 0]
    dwh_b = np.asarray(inputs['dwh_b'], np.float32)
    ww = np.asarray(inputs['dww_w'], np.float32)[:, 0, 0, :]
    dww_b = np.asarray(inputs['dww_b'], np.float32)
    pw_w = np.asarray(inputs['pw_w'], np.float32)[:, :, 0, 0]
    pw_b = np.asarray(inputs['pw_b'], np.float32)
    bn_g = np.asarray(inputs['bn_g'], np.float32)
    bn_b = np.asarray(inputs['bn_b'], np.float32)

    W_ln = W_in * ln_w[None, :]
    G = W_ln.sum(1)
    B0 = W_in @ ln_b
    conv_be = conv_b + B0[:DI] * conv_w.sum(1)

    lhsT_xz65 = np.zeros((C + 1, 2 * DI), np.float32)
    lhsT_xz65[0:C, :] = W_ln.T
    lhsT_xz65[C, DI:2 * DI] = B0[DI:]     # z-half bias row; xm bias lives in conv_be
    com = {
        "lhsT_xz": lhsT_xz65.astype(bf),
        "ones64": np.full((C, C), 1.0 / C, dtype=bf),
        "conv_diag": np.concatenate(
            [np.diag(conv_w[:, k]) for k in range(DCONV)], 1).astype(bf),
        "conv_be": conv_be[:, None],
        "lhsT_dt": W_dt.T.astype(bf),
        "b_dt": b_dt[:, None],
        "lhsT_out": W_out.T.astype(bf),
        "I128": np.eye(DI, dtype=np.float32).astype(bf),
        "sel8": _sel72().astype(bf),
        "ax5": np.concatenate([
            np.diag(wh[:, 1] + ww[:, 1]),
            np.diag(wh[:, 0]), np.diag(wh[:, 2]),
            np.diag(ww[:, 0]), np.diag(ww[:, 2])], 1).astype(bf),
        "bias_ax": (dwh_b + dww_b)[:, None],
        "lhsT_pw": pw_w.T,
        "pw_b": pw_b[:, None],
        "bn_g": bn_g[:, None],
        "bn_b": bn_b[:, None],
    }
    for k, v in com.items():
        if v.dtype == bf:
            com[k] = np.ascontiguousarray(v)
        else:
            com[k] = np.ascontiguousarray(v, dtype=np.float32)

    in_maps = []
    for c in range(8):
        b, half = c // 2, c % 2
        n0 = half * NLOC
        xp = np.zeros((DI, 72), np.float32)
        xp[:, 0:4] = W_xp[0:4].T
        xp[:, 32:40] = W_xp[[4 + n for n in range(n0, n0 + NLOC)]].T
        xp[:, 64:72] = W_xp[[4 + N + n for n in range(n0, n0 + NLOC)]].T
        m = dict(com)
        m["xb"] = np.ascontiguousarray(x[b].reshape(C, L))
        m["lhsT_xp"] = np.ascontiguousarray(xp.astype(bf))
        m["A_cols"] = np.ascontiguousarray(A[:, n0:n0 + NLOC])
        m["diag_D"] = np.ascontiguousarray(
            np.diag(D_param if half == 0 else np.zeros(DI, np.float32)).astype(bf))
        in_maps.append(m)
    return in_maps


def kernel(**inputs):
    if "prog" not in _prog_cache:
        _prog_cache["prog"] = _build_program()
    nc = _prog_cache["prog"]
    in_maps = _host_prep(inputs)
    res = bass_utils.run_bass_kernel_spmd(nc, in_maps, core_ids=list(range(8)))
    pooled = np.stack([res.results[2 * b]["pooled_o"] for b in range(B)])
    skip = np.stack([res.results[2 * b]["skip_o"].reshape(C, H, W) for b in range(B)])
    return pooled, skip
